# revision 26
# baseline (speedup 1.0000x reference)
"""Multi-head causal attention (B=2, T=2048, E=1024, H=16, D=64) on 8 trn2 cores.

Sharding: tensor-parallel over heads — core c owns heads {2c, 2c+1} (a 128-wide
slice of the hidden dim). Each core computes q/k/v projections for its heads
over the full sequence, causal attention, and a partial output projection
(contraction over its 128 rows of Wo). The host sums the 8 bf16 partials + bias.

v2 dataflow (vs the 164.4us v1): the P@V matmul is flipped so the exp'd
score block P^T[tk,tq] is the *stationary* operand (full 128x128 PE
utilisation, ap=64 per matmul) and V[tk,d] moves; the result O[tq,d] is
PE-transposed back to O^T for the output projection. Z = sum(exp) comes from
ap=1 ones-matmul chains instead of a 65th V column, and 1/Z is applied
per-partition during the O copy (tensor_scalar on DVE), killing v1's
broadcast matmul. Triangle masks run on GPSIMD (SBUF-only op) to keep DVE
for PSUM copies. exp activations are per tk-block over both heads
[128,2,512-f0], causally trimmed. Projections are emitted just-in-time as
filler units inside the attention wave stream so the PE never starves:
chunk c's waves interleave the projections needed by chunk c+1/c+2.

PSUM (8 banks): sc 2x[128,2,512] (4) + O|Z accumulators 2x[128,2,2,65] (2) +
mm [128,512] x2 (2, shared by proj / out-proj / transpose tiles).

Timing signal is concourse TimelineSim (no NTFF under this axon client).
"""

import numpy as np
import ml_dtypes
from collections import deque

import concourse.bass as bass
import concourse.tile as tile
from concourse import bacc, mybir
from concourse.bass_utils import run_bass_kernel_spmd
from concourse.masks import make_identity
from contextlib import ExitStack

B, T, E, H, D = 2, 2048, 1024, 16, 64
BT = B * T            # 4096 tokens total
NCORE = 8
KC = E // 128         # contraction chunks for projections = 8
CQ = 512              # tq chunk width
NQB = T // CQ         # tq chunks per batch = 4
NKB = T // 128        # tk blocks per batch = 16

F32 = mybir.dt.float32
BF16 = mybir.dt.bfloat16
AF = mybir.ActivationFunctionType

_cache = {}


def _build():
    nc = bacc.Bacc("TRN2", target_bir_lowering=False, debug=False,
                   num_devices=NCORE)

    xT = nc.dram_tensor("xT", [E, BT], BF16, kind="ExternalInput").ap()
    wq = nc.dram_tensor("wq", [128, E], BF16, kind="ExternalInput").ap()
    wk = nc.dram_tensor("wk", [128, E], BF16, kind="ExternalInput").ap()
    wv = nc.dram_tensor("wv", [128, E], BF16, kind="ExternalInput").ap()
    wo = nc.dram_tensor("wo", [128, E], BF16, kind="ExternalInput").ap()
    tri = nc.dram_tensor("tri", [128, 128], BF16, kind="ExternalInput").ap()
    out = nc.dram_tensor("out", [BT, E], BF16, kind="ExternalOutput").ap()

    with tile.TileContext(nc) as tc, ExitStack() as ctx:
        pers = ctx.enter_context(tc.tile_pool(name="pers", bufs=1))

        wq_sb = pers.tile([128, KC, 128], BF16, tag="wq")
        wk_sb = pers.tile([128, KC, 128], BF16, tag="wk")
        wv_sb = pers.tile([128, KC, 128], BF16, tag="wv")
        wo_sb = pers.tile([128, E], BF16, tag="wo")
        tri_sb = pers.tile([128, 128], BF16, tag="tri")
        eye_sb = pers.tile([128, 128], F32, tag="eye")
        qt_sb = pers.tile([128, BT], BF16, tag="qt")    # [dims(2 heads), tok]
        kt_sb = pers.tile([128, BT], BF16, tag="kt")
        # V natural + ones col per head: [tok%128, blk, h, d|1]; the ones
        # column makes the flipped P^T-stationary PV matmul emit Z = sum(exp)
        # as output column 64 for free.
        v_sb = pers.tile([128, BT // 128, 2, 65], BF16, tag="v")

        # wq queued first on the sync HWDGE queue so the first projection
        # matmul gates on as little DMA as possible (wk/wv/etc land while
        # the q unit runs).
        nc.sync.dma_start(wq_sb[:], wq.rearrange("p (kc d) -> p kc d", kc=KC))
        nc.vector.memset(v_sb[:, :, :, 64:65], 1.0)
        make_identity(nc, eye_sb[:])

        def load_late_weights():
            nc.sync.dma_start(wk_sb[:],
                              wk.rearrange("p (kc d) -> p kc d", kc=KC))
            nc.sync.dma_start(wv_sb[:],
                              wv.rearrange("p (kc d) -> p kc d", kc=KC))
            nc.sync.dma_start(tri_sb[:], tri[:])
            nc.sync.dma_start(wo_sb[:], wo[:])

        # SBUF pools
        xts_pool = ctx.enter_context(tc.tile_pool(name="xts", bufs=16))
        pt_pool = ctx.enter_context(tc.tile_pool(name="pt", bufs=2))
        osb_pool = ctx.enter_context(tc.tile_pool(name="osb", bufs=3))
        otsb_pool = ctx.enter_context(tc.tile_pool(name="otsb", bufs=3))
        outsb_pool = ctx.enter_context(tc.tile_pool(name="outsb", bufs=3))
        zr_pool = ctx.enter_context(tc.tile_pool(name="zr", bufs=2))

        # PSUM pools: 4 + 2 + 2 = 8 banks
        sc_pool = ctx.enter_context(tc.tile_pool(name="sc", bufs=2,
                                                 space="PSUM"))
        acc_pool = ctx.enter_context(tc.tile_pool(name="acc", bufs=1,
                                                  space="PSUM"))
        mm_pool = ctx.enter_context(tc.tile_pool(name="mm", bufs=2,
                                                 space="PSUM"))



        # ---- projection units -------------------------------------------
        def proj_pair_units(t0):
            """t0: even 512-token chunk index (0..6). Issues the pair's xT
            DMAs now; returns 6 unit callbacks (q,k,v) x (hf 0,1)."""
            xts = []
            for kc in range(KC):
                xt = xts_pool.tile([128, 2 * CQ], BF16, tag="xt",
                                   name=f"xt_{t0}_{kc}")
                nc.sync.dma_start(
                    xt[:], xT[kc * 128:(kc + 1) * 128,
                              t0 * CQ:(t0 + 2) * CQ])
                xts.append(xt)

            def qk_unit(w_sb, dst_sb, hf):
                t_ = t0 + hf
                def emit():
                    ps = mm_pool.tile([128, CQ], F32, tag="mm",
                                      name=f"qkps{t_}_{id(w_sb)}")
                    for kc in range(KC):
                        nc.tensor.matmul(
                            ps[:], w_sb[:, kc],
                            xts[kc][:, hf * CQ:(hf + 1) * CQ],
                            start=(kc == 0), stop=(kc == KC - 1))
                    nc.vector.tensor_copy(
                        dst_sb[:, t_ * CQ:(t_ + 1) * CQ], ps[:])
                return emit

            def v_unit(hf):
                t_ = t0 + hf
                def emit():
                    v_ps = mm_pool.tile([128, CQ], F32, tag="mm",
                                        name=f"vps{t_}")
                    for j in range(CQ // 128):
                        jf = hf * CQ + j * 128
                        for kc in range(KC):
                            nc.tensor.matmul(
                                v_ps[:, j * 128:(j + 1) * 128],
                                xts[kc][:, jf:jf + 128],
                                wv_sb[:, kc], start=(kc == 0),
                                stop=(kc == KC - 1))
                    b4 = t_ * (CQ // 128)
                    nc.vector.tensor_copy(
                        v_sb[:, b4:b4 + 4, :, 0:64],
                        v_ps[:].rearrange("p (j h v) -> p j h v",
                                          j=4, h=2))
                return emit

            return [qk_unit(wq_sb, qt_sb, 0), qk_unit(wk_sb, kt_sb, 0),
                    v_unit(0), qk_unit(wq_sb, qt_sb, 1),
                    qk_unit(wk_sb, kt_sb, 1), v_unit(1)]

        # ---- filler machinery -------------------------------------------
        # proj_q entries are (token_chunk, callback): the unit MUST be
        # emitted before the attention chunk that consumes that token chunk
        # (a later emission would deadlock the in-order PE queue).
        tails_q = deque()
        proj_q = deque()
        dma_pending = []   # (dram_slice, sbuf_tile): out DMAs deferred one
                           # tail so the SP queue never stalls on copy sems

        def flush_out_dma():
            while dma_pending:
                dst, src = dma_pending.pop(0)
                nc.sync.dma_start(dst, src)

        def drain_tails():
            while tails_q:
                tails_q.popleft()()

        def force_proj_upto(tc_needed):
            while proj_q and proj_q[0][0] <= tc_needed:
                proj_q.popleft()[1]()

        # ---- prologue ----------------------------------------------------
        units0 = proj_pair_units(0)
        load_late_weights()
        for u in units0[:3]:      # q,k,v for tokens 0..511
            u()
        proj_q.extend((1, u) for u in units0[3:])

        # proj pair creation / unit queueing schedule, per global chunk idx:
        #   create pair(t0) => issue its xT DMAs at that chunk's start
        #   queue: which units enter proj_q at that chunk's start
        pair_create = {1: 2, 2: 4, 3: 6}
        stash = {}

        # batch-1 chunks run [c1, c2, c3, c0] so the kernel drains on a
        # 4-block chunk (the big Act-paced c3 chunk still gets proj filler)
        chunk_list = [(0, 0), (0, 1), (0, 2), (0, 3),
                      (1, 1), (1, 2), (1, 3), (1, 0)]
        queue_map = {1: [2, 3], 2: [4], 3: [5], 4: [6], 5: [7]}
        PROJ_NS = 1707.0   # PE ns per proj unit (8 matmuls x 512 rows)
        TAIL_NS = 560.0    # PE ns per tail (transpose + 2 out-proj mm)
        deficit = 0.0

        for ci, (b, c) in enumerate(chunk_list):
            if ci in pair_create:
                units = proj_pair_units(pair_create[ci])
                stash[pair_create[ci]] = units[:3]
                stash[pair_create[ci] + 1] = units[3:]
            for t in queue_map.get(ci, []):
                proj_q.extend((t, u) for u in stash.pop(t))
            # anything this chunk's scores/PV depends on must be emitted now
            force_proj_upto(b * NQB + c)
            if ci + 1 < len(chunk_list):
                nb, ncc = chunk_list[ci + 1]
                next_need = nb * NQB + ncc
            else:
                next_need = 99

            tb = b * T
            tq0 = c * CQ
            nblk = 4 * (c + 1)
            pt = pt_pool.tile([128, NKB, 2, CQ], BF16, tag="pt",
                              name=f"pt_{b}_{c}")
            zr_sb = zr_pool.tile([128, 2, 2, 2], F32, tag="zr",
                                 name=f"zr_{b}_{c}")  # [pair, gsub, h]
            # per-chunk O|Z accumulators [tq, gsub, h, d|Z]: pool rotation
            # (bufs=1) orders the next chunk's first PV write after this
            # chunk's tail reads
            o_ps = [acc_pool.tile([128, 2, 2, 65], F32, tag=f"o{i}",
                                  name=f"o_ps{i}_{b}_{c}")
                    for i in range(2)]

            def make_tail(g, b=b, c=c, tb=tb, tq0=tq0, zr_sb=zr_sb,
                          o_ps=o_ps):
                def emit():
                    op = o_ps[g // 2]
                    gs = g % 2
                    if gs == 0:
                        # 1/Z for this tq group pair (both groups/heads: the
                        # pair's chains have both stopped by emission time)
                        nc.vector.reciprocal(
                            zr_sb[:, g // 2], op[:, :, :, 64])
                    # normalized O copy (per head, per-partition 1/Z scale)
                    o_sb = osb_pool.tile([128, 128], F32, tag="osb",
                                         name=f"osb_{b}_{c}_{g}")
                    for h in range(2):
                        nc.vector.tensor_scalar_mul(
                            o_sb[:, h * 64:(h + 1) * 64],
                            op[:, gs, h, 0:64],
                            zr_sb[:, g // 2, gs, h:h + 1])
                    # transpose O[tq,d] -> O^T[d,tq] (f32, 2 cyc/row)
                    tp = mm_pool.tile([128, 512], F32, tag="mm",
                                      name=f"tp_{b}_{c}_{g}")
                    nc.tensor.transpose(tp[:, 0:128], o_sb[:], eye_sb[:])
                    ot_sb = otsb_pool.tile([128, 128], BF16, tag="otsb",
                                           name=f"otsb_{b}_{c}_{g}")
                    nc.vector.tensor_copy(ot_sb[:], tp[:, 0:128])
                    # output projection + final copy + DMA
                    out_sb = outsb_pool.tile([128, E], BF16, tag="outsb",
                                             name=f"outsb_{b}_{c}_{g}")
                    for eh in range(2):
                        ops = mm_pool.tile([128, 512], F32, tag="mm",
                                           name=f"ops_{b}_{c}_{g}_{eh}")
                        nc.tensor.matmul(
                            ops[:], ot_sb[:],
                            wo_sb[:, eh * 512:(eh + 1) * 512],
                            start=True, stop=True)
                        if b == 0 and eh == 1:
                            nc.scalar.copy(
                                out_sb[:, eh * 512:(eh + 1) * 512], ops[:])
                        else:
                            nc.vector.tensor_copy(
                                out_sb[:, eh * 512:(eh + 1) * 512], ops[:])
                    tqg = tb + tq0 + g * 128
                    flush_out_dma()
                    dma_pending.append((out[tqg:tqg + 128, :], out_sb[:]))
                return emit

            # PSUM has_written bits: a start=True matmul clears them for the
            # WHOLE bank, so only the first PV matmul per o_ps bank per chunk
            # may use start=True. Later chains' first matmuls (kb==0,
            # start=False) overwrite-where-bit-clear, then accumulate.
            bank_started = [False, False]

            def pv_block(kb, b=b, c=c, pt=pt, o_ps=o_ps,
                         bank_started=bank_started):
                j0 = max(0, kb - 4 * c)
                for g in range(j0, NQB):
                    for h in range(2):
                        st = not bank_started[g // 2]
                        bank_started[g // 2] = True
                        nc.tensor.matmul(
                            o_ps[g // 2][:, g % 2, h, :],
                            pt[:, kb, h, g * 128:(g + 1) * 128],
                            v_sb[:, b * NKB + kb, h],
                            start=st, stop=(kb == 4 * c + g),
                            skip_group_check=True)
                j = kb - 4 * c
                if j in (1, 3):  # group pair's chains complete
                    tails_q.append(make_tail(j - 1))
                    tails_q.append(make_tail(j))
                return (NQB - j0) * 2 * 65

            for kb in range(nblk):
                f0 = max(0, 128 * (kb - 4 * c))
                sc = sc_pool.tile([128, 2, CQ], F32, tag="sc",
                                  name=f"sc_{b}_{c}_{kb}")
                tk0 = kb * 128
                for h in range(2):
                    hs = slice(h * 64, (h + 1) * 64)
                    nc.tensor.matmul(
                        sc[:, h, f0:CQ],
                        kt_sb[hs, tb + tk0:tb + tk0 + 128],
                        qt_sb[hs, tb + tq0 + f0:tb + tq0 + CQ],
                        start=True, stop=True)
                nc.scalar.activation(
                    pt[:, kb, :, f0:CQ], sc[:, :, f0:CQ],
                    AF.Exp, scale=float(D) ** -0.5)
                if kb - 4 * c >= 0:  # diagonal block: triangle mask
                    for h in range(2):
                        nc.gpsimd.tensor_mul(
                            pt[:, kb, h, f0:f0 + 128],
                            pt[:, kb, h, f0:f0 + 128], tri_sb[:])
                pv_cyc = 0
                if kb >= 1:
                    if kb == 1:
                        drain_tails()  # prev chunk's tails before 1st pv
                    pv_cyc = pv_block(kb - 1)
                # deficit-paced filler: keep the PE fed during Act-paced
                # stretches, spend queued proj/tail work exactly where the
                # per-block PE emission falls short of the exp pace.
                act_ns = (2 * (CQ - f0) + 222) / 1.2
                pe_ns = (2 * (CQ - f0) + pv_cyc) * 0.4167
                deficit += act_ns - pe_ns
                deficit = max(-4000.0, min(deficit, 6000.0))
                if proj_q and proj_q[0][0] <= next_need:
                    proj_q.popleft()[1]()   # deadline: spread 1/slot
                    deficit -= PROJ_NS
                while deficit > 400 and (tails_q or proj_q):
                    if tails_q:
                        tails_q.popleft()()
                        deficit -= TAIL_NS
                    else:
                        proj_q.popleft()[1]()
                        deficit -= PROJ_NS
            pv_block(nblk - 1)

        drain_tails()
        while proj_q:
            proj_q.popleft()[1]()
        flush_out_dma()

    nc.compile()
    return nc


def _host_prep(x, Wq, Wk, Wv, Wo):
    bf = ml_dtypes.bfloat16
    xT = np.ascontiguousarray(
        np.asarray(x, dtype=np.float32).reshape(BT, E).T).astype(bf)

    # tri[p, f] = 1 where kept (f >= p), applied to the diagonal 128x128
    # sub-block of P^T (tk on partitions, tq on free)
    p = np.arange(128)[:, None]
    f = np.arange(128)[None, :]
    tri = (f >= p).astype(bf)

    def perm(w):
        # [E, 128] -> [128p, kc, 128d] flattened: w[kc*128+p, d] -> out[p, kc, d]
        return np.ascontiguousarray(
            w.reshape(KC, 128, 128).transpose(1, 0, 2).reshape(128, E)).astype(bf)

    Wq = np.asarray(Wq, dtype=np.float32)
    Wk = np.asarray(Wk, dtype=np.float32)
    Wv = np.asarray(Wv, dtype=np.float32)
    Wo = np.asarray(Wo, dtype=np.float32)

    in_maps = []
    for c in range(NCORE):
        sl = slice(c * 128, (c + 1) * 128)
        in_maps.append({
            "xT": xT,
            "wq": perm(Wq[:, sl]),
            "wk": perm(Wk[:, sl]),
            "wv": perm(Wv[:, sl]),
            "wo": np.ascontiguousarray(Wo[sl, :]).astype(bf),
            "tri": tri,
        })
    return in_maps


def kernel(x, Wq, Wk, Wv, Wo, bo, _trace=False, _trace_kwargs=None):
    if "nc" not in _cache:
        _cache["nc"] = _build()
    nc = _cache["nc"]

    in_maps = _host_prep(x, Wq, Wk, Wv, Wo)
    kw = {}
    if _trace:
        kw = dict(trace=True, trace_cores=[0], **(_trace_kwargs or {}))
    res = run_bass_kernel_spmd(nc, in_maps, core_ids=list(range(NCORE)), **kw)
    _cache["last_result"] = res

    total = np.zeros((BT, E), dtype=np.float32)
    for r in res.results:
        total += np.asarray(r["out"], dtype=np.float32)
    total += np.asarray(bo, dtype=np.float32)[None, :]
    return total.reshape(B, T, E)
